# revision 14
# baseline (speedup 1.0000x reference)
"""Trainium2 Bass kernel for nn_CompetenceAssessor (dense_mlp, memory-bound).

Reference computation per batch element b (history [100, 2] of (success, difficulty)):
  success_rate = mean(s)                          (linear in s)
  trend        = mean(s[80:]) - mean(s[:20])      (linear in s)
  maxdiff      = max_h( (s_h > 0.5) ? d_h : 0 )   (nonlinear)
  out          = sigmoid(W3.relu(W2.relu(W1.[sr, tr, md] + b1) + b2) + b3)

Strategy (pure data-parallel over 8 cores, 32768 batch each):
  - The two linear features are folded into the first MLP layer:
      W1ext[101, 128] = [A_s @ W1[:2]; W1[2]]  so  h1pre = W1ext.T @ [s; md]
  - Stream input in 4096-batch macro tiles, one 3.3 MB DMA each, laid out as
    [128 partitions, 32 x 204 f32] rows (200 data + maxdiff slot + pad), so the
    per-sub-batch PE transpose of [s(stride 2) | md] is a single strided AP.
  - maxdiff: u2 = relu(1e12*(s-0.5)) on ScalarE (bf16 out; the huge scale makes
    the selection exact, incl. s == 0.5), z = min(u2, d) and max-reduce on
    VectorE. u2 is either 0 or >= 3e4 > d, so z is exactly d or 0.
  - MLP on the tensor engine in bf16 (full rate), N=512 moving tiles.
    mm2 col-tiles 4 groups into one PSUM bank (tile_position) so the relu+bias
    evacuation covers 4 groups per instruction.
  - Layer 3 (h2[32] . W3): |W3| is folded into W2/b2 on the host; the sign is
    folded into a signed-diagonal "identity" used by the h2 transpose matmul;
    a row-sum on VectorE then yields the pre-sigmoid in a [128, 32] shape with
    batch back on partitions, making the sigmoid cheap and the output DMA
    contiguous per partition.
"""

import os
import sys
from contextlib import ExitStack

import numpy as np

if "/opt/trn_rl_repo" not in sys.path:
    sys.path.insert(0, "/opt/trn_rl_repo")

import concourse.bass as bass
import concourse.tile as tile
from concourse import bacc, bass_utils, mybir

F32 = mybir.dt.float32
BF16 = mybir.dt.bfloat16

BATCH = 262144
HIST = 100
N_CORES = 8
BPC = BATCH // N_CORES          # 32768 batch per core
MACRO = 4096                    # batch elements per macro tile
N_MACRO = BPC // MACRO          # 8
JSUB = MACRO // 128             # 32 batch elements per partition per macro
ROWE = 204                      # row stride in f32: 200 data + 1 md + 3 pad
GROUPS = MACRO // 512           # 8 matmul groups (512 batch each)
STACKS = GROUPS // 4            # 2 h2 stacks of 4 groups

_CACHE = {}
LAST_RESULTS = None


def _build(n_macro=N_MACRO, et_on_dve=0):
    """Build + compile the SPMD Bass kernel. et_on_dve: how many of the 8
    per-macro ET psum->sbuf copies go to VectorE instead of ScalarE."""
    AL = mybir.AluOpType
    AF = mybir.ActivationFunctionType

    nc = bacc.Bacc(
        "TRN2",
        target_bir_lowering=False,
        debug=False,
        enable_asserts=False,
        num_devices=N_CORES,
    )
    bpc = n_macro * MACRO
    # 512-column shapes keep neuronx-cc happy when it compiles the XLA-side
    # dynamic_slice that shards the global input across the 8 cores.
    hist = nc.dram_tensor(
        "hist", [bpc * 200 // 512, 512], F32, kind="ExternalInput"
    ).ap()
    hist_flat = hist.rearrange("a b -> (a b)")
    w1d = nc.dram_tensor("w1ext", [101, 128], BF16, kind="ExternalInput").ap()
    w2d = nc.dram_tensor("w2p", [128, 32], BF16, kind="ExternalInput").ap()
    b1d = nc.dram_tensor("b1v", [128, 1], F32, kind="ExternalInput").ap()
    b2d = nc.dram_tensor("b2rep", [128, 1], F32, kind="ExternalInput").ap()
    b3d = nc.dram_tensor("b3v", [128, 1], F32, kind="ExternalInput").ap()
    idd = nc.dram_tensor("ident", [128, 128], F32, kind="ExternalInput").ap()
    ubd = nc.dram_tensor("u2b", [128, 1], F32, kind="ExternalInput").ap()
    sdd = nc.dram_tensor("sident", [128, 128], BF16, kind="ExternalInput").ap()
    outd = nc.dram_tensor(
        "out", [n_macro * MACRO // 512, 512], F32, kind="ExternalOutput"
    ).ap()
    out_flat = outd.rearrange("a b -> (a b)")

    with tile.TileContext(nc) as tc, ExitStack() as ctx:
        const = ctx.enter_context(tc.tile_pool(name="const", bufs=1))
        w1t = const.tile([101, 128], BF16)
        w2t = const.tile([128, 32], BF16)
        b1t = const.tile([128, 1], F32)
        b2t = const.tile([128, 1], F32)
        b3t = const.tile([128, 1], F32)
        idt = const.tile([128, 128], F32)
        sdt = const.tile([128, 128], BF16)
        ubt = const.tile([128, 1], F32)
        nc.sync.dma_start(w1t[:], w1d)
        nc.sync.dma_start(w2t[:], w2d)
        nc.sync.dma_start(b1t[:], b1d)
        nc.sync.dma_start(b2t[:], b2d)
        nc.sync.dma_start(b3t[:], b3d)
        nc.sync.dma_start(idt[:], idd)
        nc.sync.dma_start(sdt[:], sdd)
        nc.sync.dma_start(ubt[:], ubd)

        in_pool = ctx.enter_context(tc.tile_pool(name="inp", bufs=3))
        u2_pool = ctx.enter_context(tc.tile_pool(name="u2", bufs=2))
        z_pool = ctx.enter_context(tc.tile_pool(name="z", bufs=2))
        et_pool = ctx.enter_context(tc.tile_pool(name="et", bufs=3))
        h1_pool = ctx.enter_context(tc.tile_pool(name="h1", bufs=3))
        h2r_pool = ctx.enter_context(tc.tile_pool(name="h2r", bufs=2))
        col_pool = ctx.enter_context(tc.tile_pool(name="col", bufs=2))
        sig_pool = ctx.enter_context(tc.tile_pool(name="sig", bufs=2))
        etp_pool = ctx.enter_context(tc.tile_pool(name="etp", bufs=2, space="PSUM"))
        h1p_pool = ctx.enter_context(tc.tile_pool(name="h1p", bufs=2, space="PSUM"))
        h2p_pool = ctx.enter_context(tc.tile_pool(name="h2p", bufs=2, space="PSUM"))
        h2tp_pool = ctx.enter_context(tc.tile_pool(name="h2tp", bufs=2, space="PSUM"))

        for m in range(n_macro):
            ti = in_pool.tile([128, JSUB * ROWE], F32)
            tiv = ti[:].rearrange("p (j e) -> p j e", e=ROWE)
            src = hist_flat[m * MACRO * 200 : (m + 1) * MACRO * 200].rearrange(
                "(p j h) -> p j h", p=128, j=JSUB
            )
            nc.sync.dma_start(tiv[:, :, 0:200], src)

            sv = tiv[:, :, 0:200:2]   # [128, 32, 100] success
            dv = tiv[:, :, 1:200:2]   # [128, 32, 100] difficulty
            # u2 = relu(1e12*(s-0.5)): 0 if s <= 0.5, else >= ~3e4 (> any d)
            u2t = u2_pool.tile([128, JSUB * 100], BF16)
            u2v = u2t[:].rearrange("p (j h) -> p j h", h=100)
            nc.scalar.activation(u2v, sv, AF.Relu, bias=ubt[:, 0:1], scale=1e12)
            # z = min(u2, d) = d where s > 0.5 else 0 (exact)
            zt = z_pool.tile([128, JSUB * 100], F32)
            zv = zt[:].rearrange("p (j h) -> p j h", h=100)
            nc.vector.tensor_tensor(zv, u2v, dv, AL.min)
            # maxdiff into the md slot (element 200 of each 204-row)
            nc.vector.tensor_reduce(
                out=tiv[:, :, 200:201],
                in_=zv,
                axis=mybir.AxisListType.X,
                op=AL.max,
            )

            colt = col_pool.tile([128, JSUB], F32)
            for s_ in range(STACKS):
                h2p = h2p_pool.tile([128, 512], F32)
                for a in range(4):
                    g = s_ * 4 + a
                    etp = etp_pool.tile([128, 512], F32)
                    for jj in range(4):
                        j = 4 * g + jj
                        # [s_0..s_99, md] for sub-batch j: 101 elems, stride 2
                        tin = ti[:, ROWE * j : ROWE * j + 202 : 2]
                        nc.tensor.matmul(
                            etp[0:101, 128 * jj : 128 * (jj + 1)],
                            tin,
                            idt[:],
                            is_transpose=True,
                        )
                    et = et_pool.tile([128, 512], BF16)
                    if g < et_on_dve:
                        nc.vector.tensor_copy(et[0:101, :], etp[0:101, :])
                    else:
                        nc.scalar.copy(et[0:101, :], etp[0:101, :])
                    h1p = h1p_pool.tile([128, 512], F32)
                    nc.tensor.matmul(
                        h1p[:, :],
                        w1t[:],
                        et[0:101, :],
                        start=True,
                        stop=True,
                    )
                    h1s = h1_pool.tile([128, 512], BF16)
                    nc.scalar.activation(
                        h1s[:], h1p[:, :], AF.Relu, bias=b1t[:, 0:1], scale=1.0
                    )
                    nc.tensor.matmul(
                        h2p[32 * a : 32 * (a + 1), :],
                        w2t[:],
                        h1s[:],
                        start=True,
                        stop=True,
                        tile_position=(0, 32 * a),
                    )
                # h2r = relu(h2p + b2rep)  (|W3| folded into W2/b2 on host)
                h2r = h2r_pool.tile([128, 512], BF16)
                nc.scalar.activation(
                    h2r[:], h2p[:, :], AF.Relu, bias=b2t[:, 0:1], scale=1.0
                )
                # transpose h2r back to batch-on-partitions with the W3 sign
                # folded in: out = h2r_chunk.T @ diag(sign)
                h2tp = h2tp_pool.tile([128, 512], F32)
                for q in range(4):
                    nc.tensor.matmul(
                        h2tp[:, 128 * q : 128 * (q + 1)],
                        h2r[:, 128 * q : 128 * (q + 1)],
                        sdt[:],
                        start=True,
                        stop=True,
                    )
                # h2tp[p, (q, a, k)]; sum over k -> col j-16s = 4a + q
                h2tv = h2tp[:, :].rearrange("p (q a k) -> p q a k", q=4, a=4)
                cview = colt[:, 16 * s_ : 16 * s_ + 16].rearrange(
                    "p (a q) -> p q a", a=4
                )
                nc.vector.tensor_reduce(
                    out=cview,
                    in_=h2tv,
                    axis=mybir.AxisListType.X,
                    op=AL.add,
                )
            sg = sig_pool.tile([128, JSUB], F32)
            nc.scalar.activation(
                sg[:], colt[:], AF.Sigmoid, bias=b3t[:, 0:1], scale=1.0
            )
            dst = out_flat[m * MACRO : (m + 1) * MACRO].rearrange(
                "(p j) -> p j", p=128
            )
            nc.scalar.dma_start(dst, sg[:])

    nc.compile()
    return nc


def _prep_weights(W1, b1, W2, b2, W3, b3):
    import ml_dtypes

    W1 = np.asarray(W1, np.float32)
    W2 = np.asarray(W2, np.float32)
    W3 = np.asarray(W3, np.float32)
    b1 = np.asarray(b1, np.float32)
    b2 = np.asarray(b2, np.float32)
    b3 = np.asarray(b3, np.float32)
    A = np.zeros((100, 2), np.float32)
    A[:, 0] = 1.0 / 100.0
    A[80:, 1] += 1.0 / 20.0
    A[:20, 1] -= 1.0 / 20.0
    w1ext = np.concatenate([A @ W1[:2], W1[2:3, :]], axis=0)
    absw3 = np.abs(W3[:, 0])
    sgn = np.sign(W3[:, 0]).astype(np.float32)
    w2p = W2 * absw3[None, :]
    b2p = (b2 * absw3).astype(np.float32)
    sident = np.diag(np.tile(sgn, 4)).astype(np.float32)
    return {
        "w1ext": np.ascontiguousarray(w1ext.astype(ml_dtypes.bfloat16)),
        "w2p": np.ascontiguousarray(w2p.astype(ml_dtypes.bfloat16)),
        "b1v": np.ascontiguousarray(b1[:, None].astype(np.float32)),
        "b2rep": np.ascontiguousarray(np.tile(b2p, 4)[:, None]),
        "b3v": np.full((128, 1), float(b3[0]), np.float32),
        "ident": np.eye(128, dtype=np.float32),
        "u2b": np.full((128, 1), -5e11, np.float32),
        "sident": np.ascontiguousarray(sident.astype(ml_dtypes.bfloat16)),
    }


def kernel(performance_history, W1, b1, W2, b2, W3, b3):
    global LAST_RESULTS
    hist = np.asarray(performance_history, np.float32).reshape(BATCH, 2 * HIST)
    wmap = _prep_weights(W1, b1, W2, b2, W3, b3)

    if "nc" not in _CACHE:
        _CACHE["nc"] = _build()
    nc = _CACHE["nc"]

    in_maps = []
    for c in range(N_CORES):
        m = {
            "hist": np.ascontiguousarray(hist[c * BPC : (c + 1) * BPC]).reshape(
                BPC * 200 // 512, 512
            )
        }
        m.update(wmap)
        in_maps.append(m)

    res = bass_utils.run_bass_kernel_spmd(nc, in_maps, core_ids=list(range(N_CORES)))
    LAST_RESULTS = res
    out = np.concatenate(
        [res.results[c]["out"].reshape(BPC) for c in range(N_CORES)], axis=0
    )
    return out.astype(np.float32)
